# revision 7
# baseline (speedup 1.0000x reference)
"""MinGRU cell on 8 Trainium2 NeuronCores (Bass/Tile).

Math (per batch b, hidden h):
    gz = x @ W_z^T ; gh = x @ W_h^T                 (two GEMMs, K=D=1024)
    z  = sigmoid(gz + b_z)
    h_t = (1 - z_t) * h_{t-1} + z_t * (gh_t + b_h)  (affine scan over T)

Distribution: data-parallel over batch B=16 -> 2 batches per core, weights
replicated; no cross-core communication.

Per-core steady state: the PE streams the GEMMs (128 N=512 bf16 matmuls
per 512-token step, ~218 ns each = the bf16 roofline) with no other PE
work: the scan output is stored to DRAM in [H, T] layout straight from
the DVE scan tiles and transposed to [T, H] on the host, so the PE's
out-transposes (and their ACT/DVE copies and the 16 us drain tail) are
gone entirely. x^T tiles for steps 1..7 are produced by the DMA crossbar
(dma_start_transpose) straight from (host-precast bf16) DRAM, issued a
full step ahead — the crossbar's completion semaphore has been observed
to lead its data on profiled runs, so every crossbar transpose here has
~10+ us between data landing and first consumer. Step 0's x^T is built
on the PE instead (plain loads + tensor-engine transposes, j-outer so
transposes start as soon as each x row-block lands). W arrives
pre-transposed bf16 from the host (weight pre-packing) in four half-H
strided DMAs per W; step-0 GEMMs are ordered in z/h quarter-pairs so
they track the W quarters as they land on the two HWDGE queues (W_z on
SP, W_h on ACT). Bias/h0 gathers go to the GpSimd SWDGE queue. ACT runs
the two sigmoids (z and 1-z), DVE the (gh+b_h)*z fuse and the affine
scan. Output is written bf16 and upcast to f32 on the host (it was
computed in bf16 either way).
"""

import sys

sys.path.insert(0, "/opt/trn_rl_repo")

from contextlib import ExitStack

import numpy as np
import ml_dtypes

import concourse.bass as bass
import concourse.mybir as mybir
import concourse.tile as tile
from concourse import bacc
from concourse.bass import ts, ds
from concourse.bass_utils import run_bass_kernel_spmd
from concourse.masks import make_identity

B, T, D, H = 16, 2048, 1024, 1024
NCORES = 8
B_LOC = B // NCORES  # 2
P = 128
TC = 512  # tokens per step
NSTEP = B_LOC * T // TC  # 8
NTC = T // TC  # 4 steps per batch
TSUB = TC // P  # 4
DC = D // P  # 8 contraction chunks
HC = H // P  # 8 hidden chunks
HQ = H // 4  # 256, one h-quarter of W per DMA

F32 = mybir.dt.float32
BF16 = mybir.dt.bfloat16
AF = mybir.ActivationFunctionType
OP = mybir.AluOpType

_CACHE = {}


def _mingru_tile(tc, out, x, h0, wzT, bz, whT, bh):
    nc = tc.nc

    with ExitStack() as ctx:
        consts = ctx.enter_context(tc.tile_pool(name="consts", bufs=1))

        id_bf = consts.tile([P, P], BF16)
        make_identity(nc, id_bf)

        # Small strided gathers on the otherwise idle SWDGE queue.
        bz_sb = consts.tile([P, HC], F32)
        nc.gpsimd.dma_start(out=bz_sb, in_=bz.rearrange("(c p) -> p c", p=P))
        bh_sb = consts.tile([P, HC], F32)
        nc.gpsimd.dma_start(out=bh_sb, in_=bh.rearrange("(c p) -> p c", p=P))
        hp_sb = consts.tile([P, B_LOC * HC], F32)
        nc.gpsimd.dma_start(out=hp_sb, in_=h0.rearrange("b (c p) -> p (b c)", p=P))
        nbz_sb = consts.tile([P, HC], F32)
        nc.vector.tensor_scalar_mul(nbz_sb, bz_sb, -1.0)

        xt_p = ctx.enter_context(tc.tile_pool(name="xt", bufs=2))
        azb_p = ctx.enter_context(tc.tile_pool(name="azb", bufs=2))
        scan_p = ctx.enter_context(tc.tile_pool(name="scan", bufs=2))
        xnat_p = ctx.enter_context(tc.tile_pool(name="xnat", bufs=1))

        def step_bt(s):
            return s // NTC, s % NTC

        # W^T arrives pre-transposed [D, H] bf16 from the host. One strided
        # DMA per h-quarter (512B row chunks):
        #   wt[wn][r][p, dc*HQ + h'] = W^T[dc*128 + p, r*HQ + h']
        # lhsT block (hc,dc) = wt[wn][hc//2][:, dc*HQ + (hc%2)*128 ...].
        # Queue plan: the z/h quarter-0 pair leads the SP queue (first
        # GEMMs wait only on it), step-0's natural-layout x leads the ACT
        # queue (it gates the PE transpose prologue), and the remaining
        # quarters split z->SP / h->ACT so each quarter pair lands a few
        # microseconds before the step-0 quarter-pair GEMMs consume it.
        wt = {"z": [], "h": []}

        def w_load(wn, r, eng):
            w_ap = wzT if wn == "z" else whT
            w_sb = consts.tile([P, DC * HQ], BF16, name=f"wt_{wn}{r}")
            eng.dma_start(
                out=w_sb.rearrange("p (dc h) -> p dc h", h=HQ),
                in_=w_ap[:, ds(r * HQ, HQ)].rearrange("(dc p) h -> p dc h", p=P),
            )
            wt[wn].append(w_sb)

        w_load("z", 0, nc.sync)
        w_load("h", 0, nc.sync)

        xn0 = []
        for j in range(TSUB):
            t_ = xnat_p.tile([P, D], BF16, tag=f"xn{j}", name=f"xn0_{j}")
            nc.scalar.dma_start(out=t_, in_=x[0, ds(j * P, P), :])
            xn0.append(t_)

        for r in range(1, 4):
            w_load("z", r, nc.sync)
        for r in range(1, 4):
            w_load("h", r, nc.scalar)

        xts = {}

        def t_x(s):  # crossbar transpose, issued a full step ahead of use
            b, tci = step_bt(s)
            tiles = []
            for dc in range(DC):
                t_ = xt_p.tile([P, TC], BF16, tag=f"xt{dc}", name=f"xt_{s}_{dc}")
                nc.sync.dma_start_transpose(t_, x[b, ds(tci * TC, TC), ts(dc, P)])
                tiles.append(t_)
            xts[s] = tiles

        t_x(1)

        # Prologue PE work: HAM warmup junk, then step 0's x^T on the PE
        # (transpose to PSUM j-outer in two 4-dc waves so work starts as
        # each xn row-block lands; ACT/DVE alternate the PSUM->SBUF copies).
        xts[0] = []
        with tc.tile_pool(name="warm", bufs=1, space="PSUM") as warm_p, \
             tc.tile_pool(name="pxt", bufs=1, space="PSUM") as pxt_p, \
             tc.tile_pool(name="wdram", bufs=1, space="DRAM") as wdram_p:
            junk_ps = warm_p.tile([P, P], F32, name="junk_ps")
            NWARM = 30
            for i in range(NWARM):
                nc.tensor.matmul(
                    junk_ps, id_bf, id_bf, start=(i == 0), stop=(i == NWARM - 1)
                )
            junk_sb = consts.tile([P, P], F32, name="junk_sb")
            nc.vector.tensor_copy(junk_sb, junk_ps)
            junk_dr = wdram_p.tile([P, P], F32, name="junk_dr")
            nc.sync.dma_start(out=junk_dr, in_=junk_sb)

            for wave in range(2):
                pxts = [
                    pxt_p.tile([P, TC], BF16, tag=f"pxt{k}", name=f"pxt_{wave}_{k}")
                    for k in range(4)
                ]
                for j in range(TSUB):
                    for k in range(4):
                        dc = wave * 4 + k
                        nc.tensor.transpose(
                            pxts[k][:, ts(j, P)], xn0[j][:, ts(dc, P)], id_bf
                        )
                for k in range(4):
                    dc = wave * 4 + k
                    xt_sb = xt_p.tile([P, TC], BF16, tag=f"xt{dc}", name=f"xt_0_{dc}")
                    eng = nc.scalar if k % 2 else nc.vector
                    if k % 2:
                        nc.scalar.copy(xt_sb, pxts[k])
                    else:
                        nc.vector.tensor_copy(xt_sb, pxts[k])
                    xts[0].append(xt_sb)

        # PSUM: 4 z + 4 h GEMM banks (prologue banks are re-used once the
        # ACT/DVE copies above have drained — before the first GEMM needs
        # them).
        pz_p = ctx.enter_context(tc.tile_pool(name="pz", bufs=4, space="PSUM"))
        ph_p = ctx.enter_context(tc.tile_pool(name="ph", bufs=4, space="PSUM"))

        scans = {}

        def gemm(s, hc, wn):
            pool = pz_p if wn == "z" else ph_p
            psum = pool.tile([P, TC], F32, tag="p" + wn, name=f"ps{wn}_{s}_{hc}")
            xt = xts[s]
            w_sb = wt[wn][hc // 2]
            for dc in range(DC):
                nc.tensor.matmul(
                    psum,
                    w_sb[:, ds(dc * HQ + (hc % 2) * P, P)],
                    xt[dc],
                    start=(dc == 0),
                    stop=(dc == DC - 1),
                )
            return psum

        def post(s, hc, psum_z, psum_h):
            b, tci = step_bt(s)
            a_sb = azb_p.tile([P, TC], BF16, tag="a", name=f"a_{s}_{hc}")
            nc.scalar.activation(
                a_sb, psum_z, AF.Sigmoid, bias=nbz_sb[:, hc : hc + 1], scale=-1.0
            )
            z_sb = azb_p.tile([P, TC], F32, tag="z", name=f"z_{s}_{hc}")
            nc.scalar.activation(
                z_sb, psum_z, AF.Sigmoid, bias=bz_sb[:, hc : hc + 1], scale=1.0
            )
            bsc = azb_p.tile([P, TC], BF16, tag="b", name=f"b_{s}_{hc}")
            nc.vector.scalar_tensor_tensor(
                bsc, psum_h, bh_sb[:, hc : hc + 1], z_sb, op0=OP.add, op1=OP.mult
            )
            # bf16 scan output: the scan accumulator is fp32 in HW
            # regardless of out dtype, so only stored values round; bf16
            # halves the store bytes.
            sc = scan_p.tile([P, TC], BF16, tag=f"sc{hc}", name=f"sc_{s}_{hc}")
            if tci == 0:
                init = hp_sb[:, b * HC + hc : b * HC + hc + 1]
            else:
                init = scans[s - 1][hc][:, TC - 1 : TC]
            nc.vector.tensor_tensor_scan(sc, a_sb, bsc, init, op0=OP.mult, op1=OP.add)
            scans.setdefault(s, [None] * HC)[hc] = sc
            # Store the scan tile directly in [H, T] layout on the SP
            # HWDGE queue (host transposes). The store's scan-semaphore
            # wait must NOT sit on the ACT engine: it would serialize the
            # sigmoids (which drain the GEMM PSUM banks) behind the DVE
            # scan chain and stall the PE. The SP engine only issues the
            # next-next step's crossbar loads after these, which is safe.
            nc.sync.dma_start(
                out=out[b, ds(hc * P, P), ds(tci * TC, TC)], in_=sc
            )

        # --- steps -------------------------------------------------------
        for s in range(NSTEP):
            if s == 0:
                # GEMMs in z/h quarter-pairs so they track the W quarter
                # DMAs landing on the two queues.
                for r in range(4):
                    pzs = [gemm(0, 2 * r + i, "z") for i in range(2)]
                    phs = [gemm(0, 2 * r + i, "h") for i in range(2)]
                    for i in range(2):
                        post(0, 2 * r + i, pzs[i], phs[i])
            else:
                if s + 1 < NSTEP:
                    t_x(s + 1)
                for hc in range(HC):
                    psum_z = gemm(s, hc, "z")
                    psum_h = gemm(s, hc, "h")
                    post(s, hc, psum_z, psum_h)
                if s - 2 in scans:
                    del scans[s - 2]


def build():
    if "nc" in _CACHE:
        return _CACHE["nc"]
    nc = bacc.Bacc(
        "TRN2", target_bir_lowering=False, debug=False, num_devices=NCORES
    )
    x = nc.dram_tensor("x", [B_LOC, T, D], BF16, kind="ExternalInput").ap()
    h0 = nc.dram_tensor("h0", [B_LOC, H], F32, kind="ExternalInput").ap()
    wzT = nc.dram_tensor("wzT", [D, H], BF16, kind="ExternalInput").ap()
    bz = nc.dram_tensor("bz", [H], F32, kind="ExternalInput").ap()
    whT = nc.dram_tensor("whT", [D, H], BF16, kind="ExternalInput").ap()
    bh = nc.dram_tensor("bh", [H], F32, kind="ExternalInput").ap()
    out = nc.dram_tensor("out", [B_LOC, H, T], BF16, kind="ExternalOutput").ap()
    with tile.TileContext(nc) as tctx:
        _mingru_tile(tctx, out, x, h0, wzT, bz, whT, bh)
    nc.compile()
    _CACHE["nc"] = nc
    return nc


def make_in_maps(x, h_prev, W_z, b_z, W_h, b_h):
    x = np.asarray(x, dtype=np.float32).astype(ml_dtypes.bfloat16)
    h_prev = np.ascontiguousarray(np.asarray(h_prev, dtype=np.float32))
    wzT = np.asarray(W_z, dtype=np.float32).T.astype(ml_dtypes.bfloat16)
    whT = np.asarray(W_h, dtype=np.float32).T.astype(ml_dtypes.bfloat16)
    b_z = np.ascontiguousarray(np.asarray(b_z, dtype=np.float32))
    b_h = np.ascontiguousarray(np.asarray(b_h, dtype=np.float32))
    in_maps = []
    for c in range(NCORES):
        sl = slice(c * B_LOC, (c + 1) * B_LOC)
        in_maps.append(
            {
                "x": np.ascontiguousarray(x[sl]),
                "h0": h_prev[sl],
                "wzT": wzT,
                "bz": b_z,
                "whT": whT,
                "bh": b_h,
            }
        )
    return in_maps


def kernel(x, h_prev, W_z, b_z, W_h, b_h, trace=False):
    nc = build()
    in_maps = make_in_maps(x, h_prev, W_z, b_z, W_h, b_h)
    res = run_bass_kernel_spmd(
        nc, in_maps, core_ids=list(range(NCORES)), trace=trace
    )
    # Device output is [B_loc, H, T] bf16; transpose to (B, T, H) f32 here.
    out = np.concatenate(
        [
            np.asarray(r["out"]).astype(np.float32).transpose(0, 2, 1)
            for r in res.results
        ],
        axis=0,
    )
    if trace:
        _CACHE["last_results"] = res
    return out


# revision 9
# speedup vs baseline: 1.1303x; 1.1303x over previous
"""MinGRU cell on 8 Trainium2 NeuronCores (Bass/Tile).

Math (per batch b, hidden h):
    gz = x @ W_z^T ; gh = x @ W_h^T                 (two GEMMs, K=D=1024)
    z  = sigmoid(gz + b_z)
    h_t = (1 - z_t) * h_{t-1} + z_t * (gh_t + b_h)  (affine scan over T)

Distribution: data-parallel over batch B=16 -> 2 batches per core, weights
replicated; no cross-core communication.

Per-core steady state: the PE streams the GEMMs (128 N=512 bf16 matmuls
per 512-token step, ~218 ns each = the bf16 roofline) with no other PE
work: the scan output is stored to DRAM in [H, T] layout straight from
the DVE scan tiles and transposed to [T, H] on the host, so the PE's
out-transposes (and their ACT/DVE copies and the 16 us drain tail) are
gone entirely. x^T tiles for steps 1..7 are produced by the DMA crossbar
(dma_start_transpose) straight from (host-precast bf16) DRAM, issued a
full step ahead — the crossbar's completion semaphore has been observed
to lead its data on profiled runs, so every crossbar transpose here has
~10+ us between data landing and first consumer. Step 0's x^T is built
on the PE instead (plain loads + tensor-engine transposes, j-outer so
transposes start as soon as each x row-block lands). W arrives
pre-transposed bf16 from the host (weight pre-packing) in four half-H
strided DMAs per W; step-0 GEMMs are ordered in z/h quarter-pairs so
they track the W quarters as they land on the two HWDGE queues (W_z on
SP, W_h on ACT). Bias/h0 gathers go to the GpSimd SWDGE queue. ACT runs
the two sigmoids (z and 1-z), DVE the (gh+b_h)*z fuse and the affine
scan. Output is written bf16 and upcast to f32 on the host (it was
computed in bf16 either way).
"""

import sys

sys.path.insert(0, "/opt/trn_rl_repo")

from contextlib import ExitStack

import numpy as np
import ml_dtypes

import concourse.bass as bass
import concourse.mybir as mybir
import concourse.tile as tile
from concourse import bacc
from concourse.bass import ts, ds
from concourse.bass_utils import run_bass_kernel_spmd
from concourse.masks import make_identity

B, T, D, H = 16, 2048, 1024, 1024
NCORES = 8
B_LOC = B // NCORES  # 2
P = 128
TC = 512  # tokens per step
NSTEP = B_LOC * T // TC  # 8
NTC = T // TC  # 4 steps per batch
TSUB = TC // P  # 4
DC = D // P  # 8 contraction chunks
HC = H // P  # 8 hidden chunks
HQ = H // 4  # 256, one h-quarter of W per DMA

F32 = mybir.dt.float32
BF16 = mybir.dt.bfloat16
AF = mybir.ActivationFunctionType
OP = mybir.AluOpType

_CACHE = {}


def _mingru_tile(tc, out, x, h0, wzT, bz, whT, bh):
    nc = tc.nc

    with ExitStack() as ctx:
        consts = ctx.enter_context(tc.tile_pool(name="consts", bufs=1))

        id_bf = consts.tile([P, P], BF16)
        make_identity(nc, id_bf)

        # Small strided gathers on the otherwise idle SWDGE queue.
        bz_sb = consts.tile([P, HC], F32)
        nc.gpsimd.dma_start(out=bz_sb, in_=bz.rearrange("(c p) -> p c", p=P))
        bh_sb = consts.tile([P, HC], F32)
        nc.gpsimd.dma_start(out=bh_sb, in_=bh.rearrange("(c p) -> p c", p=P))
        hp_sb = consts.tile([P, B_LOC * HC], F32)
        nc.gpsimd.dma_start(out=hp_sb, in_=h0.rearrange("b (c p) -> p (b c)", p=P))
        nbz_sb = consts.tile([P, HC], F32)
        nc.vector.tensor_scalar_mul(nbz_sb, bz_sb, -1.0)

        xt_p = ctx.enter_context(tc.tile_pool(name="xt", bufs=2))
        azb_p = ctx.enter_context(tc.tile_pool(name="azb", bufs=2))
        scan_p = ctx.enter_context(tc.tile_pool(name="scan", bufs=2))
        xnat_p = ctx.enter_context(tc.tile_pool(name="xnat", bufs=1))

        def step_bt(s):
            return s // NTC, s % NTC

        # SBUF tile creation order below is LOAD-BEARING: reordering the
        # consts-pool tiles was measured to slow every PE instruction by
        # ~20% (SBUF line/bank interaction between the LDWEIGHTS stream
        # and the matmul rhs stream). Tiles are therefore created in the
        # measured-good order (xn row-blocks first, W quarters interleaved
        # z/h), and the DMA *issue* order is chosen separately below.
        xn0 = [
            xnat_p.tile([P, D], BF16, tag=f"xn{j}", name=f"xn0_{j}")
            for j in range(TSUB)
        ]

        # W^T arrives pre-transposed [D, H] bf16 from the host. One strided
        # DMA per h-quarter (512B row chunks):
        #   wt[wn][r][p, dc*HQ + h'] = W^T[dc*128 + p, r*HQ + h']
        # lhsT block (hc,dc) = wt[wn][hc//2][:, dc*HQ + (hc%2)*128 ...].
        wt = {"z": [], "h": []}
        for r in range(4):
            for wn in ("z", "h"):
                wt[wn].append(
                    consts.tile([P, DC * HQ], BF16, name=f"wt_{wn}{r}")
                )

        def w_issue(wn, r, eng):
            w_ap = wzT if wn == "z" else whT
            eng.dma_start(
                out=wt[wn][r].rearrange("p (dc h) -> p dc h", h=HQ),
                in_=w_ap[:, ds(r * HQ, HQ)].rearrange("(dc p) h -> p dc h", p=P),
            )

        # Queue plan (two ~142 GB/s HWDGE queues, SP and ACT): the z
        # quarter-0 leads SP so the first GEMMs wait only on 0.5 MB; x's
        # step-0 row-blocks lead ACT (they gate the PE transpose
        # prologue, one block rides SP); the remaining quarters stream
        # z->SP / h->ACT, each landing a few us before the step-0
        # quarter-pair GEMMs consume it.
        w_issue("z", 0, nc.sync)
        for j in range(3):
            nc.scalar.dma_start(out=xn0[j], in_=x[0, ds(j * P, P), :])
        nc.sync.dma_start(out=xn0[3], in_=x[0, ds(3 * P, P), :])
        for r in range(1, 4):
            w_issue("z", r, nc.sync)
        for r in range(4):
            w_issue("h", r, nc.scalar)

        xts = {}

        def t_x(s):  # crossbar transpose, issued a full step ahead of use
            b, tci = step_bt(s)
            tiles = []
            for dc in range(DC):
                t_ = xt_p.tile([P, TC], BF16, tag=f"xt{dc}", name=f"xt_{s}_{dc}")
                nc.sync.dma_start_transpose(t_, x[b, ds(tci * TC, TC), ts(dc, P)])
                tiles.append(t_)
            xts[s] = tiles

        t_x(1)

        # Prologue PE work: HAM warmup junk, then step 0's x^T on the PE
        # (transpose to PSUM j-outer in two 4-dc waves so work starts as
        # each xn row-block lands; ACT/DVE alternate the PSUM->SBUF copies).
        xts[0] = []
        with tc.tile_pool(name="warm", bufs=1, space="PSUM") as warm_p, \
             tc.tile_pool(name="pxt", bufs=1, space="PSUM") as pxt_p, \
             tc.tile_pool(name="wdram", bufs=1, space="DRAM") as wdram_p:
            junk_ps = warm_p.tile([P, P], F32, name="junk_ps")
            NWARM = 30
            for i in range(NWARM):
                nc.tensor.matmul(
                    junk_ps, id_bf, id_bf, start=(i == 0), stop=(i == NWARM - 1)
                )
            junk_sb = consts.tile([P, P], F32, name="junk_sb")
            nc.vector.tensor_copy(junk_sb, junk_ps)
            junk_dr = wdram_p.tile([P, P], F32, name="junk_dr")
            nc.sync.dma_start(out=junk_dr, in_=junk_sb)

            for wave in range(2):
                pxts = [
                    pxt_p.tile([P, TC], BF16, tag=f"pxt{k}", name=f"pxt_{wave}_{k}")
                    for k in range(4)
                ]
                for j in range(TSUB):
                    for k in range(4):
                        dc = wave * 4 + k
                        nc.tensor.transpose(
                            pxts[k][:, ts(j, P)], xn0[j][:, ts(dc, P)], id_bf
                        )
                for k in range(4):
                    dc = wave * 4 + k
                    xt_sb = xt_p.tile([P, TC], BF16, tag=f"xt{dc}", name=f"xt_0_{dc}")
                    eng = nc.scalar if k % 2 else nc.vector
                    if k % 2:
                        nc.scalar.copy(xt_sb, pxts[k])
                    else:
                        nc.vector.tensor_copy(xt_sb, pxts[k])
                    xts[0].append(xt_sb)

        # PSUM: 4 z + 4 h GEMM banks (prologue banks are re-used once the
        # ACT/DVE copies above have drained — before the first GEMM needs
        # them).
        pz_p = ctx.enter_context(tc.tile_pool(name="pz", bufs=4, space="PSUM"))
        ph_p = ctx.enter_context(tc.tile_pool(name="ph", bufs=4, space="PSUM"))

        scans = {}

        def gemm(s, hc, wn):
            pool = pz_p if wn == "z" else ph_p
            psum = pool.tile([P, TC], F32, tag="p" + wn, name=f"ps{wn}_{s}_{hc}")
            xt = xts[s]
            w_sb = wt[wn][hc // 2]
            for dc in range(DC):
                nc.tensor.matmul(
                    psum,
                    w_sb[:, ds(dc * HQ + (hc % 2) * P, P)],
                    xt[dc],
                    start=(dc == 0),
                    stop=(dc == DC - 1),
                )
            return psum

        def post(s, hc, psum_z, psum_h):
            b, tci = step_bt(s)
            a_sb = azb_p.tile([P, TC], BF16, tag="a", name=f"a_{s}_{hc}")
            nc.scalar.activation(
                a_sb, psum_z, AF.Sigmoid, bias=nbz_sb[:, hc : hc + 1], scale=-1.0
            )
            z_sb = azb_p.tile([P, TC], F32, tag="z", name=f"z_{s}_{hc}")
            nc.scalar.activation(
                z_sb, psum_z, AF.Sigmoid, bias=bz_sb[:, hc : hc + 1], scale=1.0
            )
            bsc = azb_p.tile([P, TC], BF16, tag="b", name=f"b_{s}_{hc}")
            nc.vector.scalar_tensor_tensor(
                bsc, psum_h, bh_sb[:, hc : hc + 1], z_sb, op0=OP.add, op1=OP.mult
            )
            # bf16 scan output: the scan accumulator is fp32 in HW
            # regardless of out dtype, so only stored values round; bf16
            # halves the store bytes.
            sc = scan_p.tile([P, TC], BF16, tag=f"sc{hc}", name=f"sc_{s}_{hc}")
            if tci == 0:
                init = hp_sb[:, b * HC + hc : b * HC + hc + 1]
            else:
                init = scans[s - 1][hc][:, TC - 1 : TC]
            nc.vector.tensor_tensor_scan(sc, a_sb, bsc, init, op0=OP.mult, op1=OP.add)
            scans.setdefault(s, [None] * HC)[hc] = sc
            # Store the scan tile directly in [H, T] layout on the SP
            # HWDGE queue (host transposes). The store's scan-semaphore
            # wait must NOT sit on the ACT engine: it would serialize the
            # sigmoids (which drain the GEMM PSUM banks) behind the DVE
            # scan chain and stall the PE. The SP engine only issues the
            # next-next step's crossbar loads after these, which is safe.
            # Last step only: ACT has no future sigmoids to poison, so
            # split the final stores across both queues to drain faster.
            eng = nc.scalar if (s == NSTEP - 1 and hc % 2) else nc.sync
            eng.dma_start(
                out=out[b, ds(hc * P, P), ds(tci * TC, TC)], in_=sc
            )

        # --- steps -------------------------------------------------------
        for s in range(NSTEP):
            if s == 0:
                # GEMMs in z/h quarter-pairs so they track the W quarter
                # DMAs landing on the two queues.
                for r in range(4):
                    pzs = [gemm(0, 2 * r + i, "z") for i in range(2)]
                    phs = [gemm(0, 2 * r + i, "h") for i in range(2)]
                    for i in range(2):
                        post(0, 2 * r + i, pzs[i], phs[i])
            else:
                if s + 1 < NSTEP:
                    t_x(s + 1)
                for hc in range(HC):
                    psum_z = gemm(s, hc, "z")
                    psum_h = gemm(s, hc, "h")
                    post(s, hc, psum_z, psum_h)
                if s - 2 in scans:
                    del scans[s - 2]


def build():
    if "nc" in _CACHE:
        return _CACHE["nc"]
    nc = bacc.Bacc(
        "TRN2", target_bir_lowering=False, debug=False, num_devices=NCORES
    )
    x = nc.dram_tensor("x", [B_LOC, T, D], BF16, kind="ExternalInput").ap()
    h0 = nc.dram_tensor("h0", [B_LOC, H], F32, kind="ExternalInput").ap()
    wzT = nc.dram_tensor("wzT", [D, H], BF16, kind="ExternalInput").ap()
    bz = nc.dram_tensor("bz", [H], F32, kind="ExternalInput").ap()
    whT = nc.dram_tensor("whT", [D, H], BF16, kind="ExternalInput").ap()
    bh = nc.dram_tensor("bh", [H], F32, kind="ExternalInput").ap()
    out = nc.dram_tensor("out", [B_LOC, H, T], BF16, kind="ExternalOutput").ap()
    with tile.TileContext(nc) as tctx:
        _mingru_tile(tctx, out, x, h0, wzT, bz, whT, bh)
    nc.compile()
    _CACHE["nc"] = nc
    return nc


def make_in_maps(x, h_prev, W_z, b_z, W_h, b_h):
    x = np.asarray(x, dtype=np.float32).astype(ml_dtypes.bfloat16)
    h_prev = np.ascontiguousarray(np.asarray(h_prev, dtype=np.float32))
    wzT = np.asarray(W_z, dtype=np.float32).T.astype(ml_dtypes.bfloat16)
    whT = np.asarray(W_h, dtype=np.float32).T.astype(ml_dtypes.bfloat16)
    b_z = np.ascontiguousarray(np.asarray(b_z, dtype=np.float32))
    b_h = np.ascontiguousarray(np.asarray(b_h, dtype=np.float32))
    in_maps = []
    for c in range(NCORES):
        sl = slice(c * B_LOC, (c + 1) * B_LOC)
        in_maps.append(
            {
                "x": np.ascontiguousarray(x[sl]),
                "h0": h_prev[sl],
                "wzT": wzT,
                "bz": b_z,
                "whT": whT,
                "bh": b_h,
            }
        )
    return in_maps


def kernel(x, h_prev, W_z, b_z, W_h, b_h, trace=False):
    nc = build()
    in_maps = make_in_maps(x, h_prev, W_z, b_z, W_h, b_h)
    res = run_bass_kernel_spmd(
        nc, in_maps, core_ids=list(range(NCORES)), trace=trace
    )
    # Device output is [B_loc, H, T] bf16; transpose to (B, T, H) f32 here.
    out = np.concatenate(
        [
            np.asarray(r["out"]).astype(np.float32).transpose(0, 2, 1)
            for r in res.results
        ],
        axis=0,
    )
    if trace:
        _CACHE["last_results"] = res
    return out


# revision 10
# speedup vs baseline: 1.1835x; 1.0470x over previous
"""MinGRU cell on 8 Trainium2 NeuronCores (Bass/Tile).

Math (per batch b, hidden h):
    gz = x @ W_z^T ; gh = x @ W_h^T                 (two GEMMs, K=D=1024)
    z  = sigmoid(gz + b_z)
    h_t = (1 - z_t) * h_{t-1} + z_t * (gh_t + b_h)  (affine scan over T)

Distribution: data-parallel over batch B=16 -> 2 batches per core, weights
replicated; no cross-core communication.

Per-core steady state: the PE streams the GEMMs (128 N=512 bf16 matmuls
per 512-token step, ~218 ns each = the bf16 roofline) with no other PE
work: the scan output is stored to DRAM in [H, T] layout straight from
the DVE scan tiles and transposed to [T, H] on the host, so the PE's
out-transposes (and their ACT/DVE copies and the 16 us drain tail) are
gone entirely. x^T tiles for steps 1..7 are produced by the DMA crossbar
(dma_start_transpose) straight from (host-precast bf16) DRAM, issued a
full step ahead — the crossbar's completion semaphore has been observed
to lead its data on profiled runs, so every crossbar transpose here has
~10+ us between data landing and first consumer. Step 0's x^T is built
on the PE instead (plain loads + tensor-engine transposes, j-outer so
transposes start as soon as each x row-block lands). W arrives
pre-transposed bf16 from the host (weight pre-packing) in four half-H
strided DMAs per W; step-0 GEMMs are ordered in z/h quarter-pairs so
they track the W quarters as they land on the two HWDGE queues (W_z on
SP, W_h on ACT). Bias/h0 gathers go to the GpSimd SWDGE queue. ACT runs
the two sigmoids (z and 1-z), DVE the (gh+b_h)*z fuse and the affine
scan. Output is written bf16 and upcast to f32 on the host (it was
computed in bf16 either way).
"""

import sys

sys.path.insert(0, "/opt/trn_rl_repo")

from contextlib import ExitStack

import numpy as np
import ml_dtypes

import concourse.bass as bass
import concourse.mybir as mybir
import concourse.tile as tile
from concourse import bacc
from concourse.bass import ts, ds
from concourse.bass_utils import run_bass_kernel_spmd
from concourse.masks import make_identity

B, T, D, H = 16, 2048, 1024, 1024
NCORES = 8
B_LOC = B // NCORES  # 2
P = 128
TC = 512  # tokens per step
NSTEP = B_LOC * T // TC  # 8
NTC = T // TC  # 4 steps per batch
TSUB = TC // P  # 4
DC = D // P  # 8 contraction chunks
HC = H // P  # 8 hidden chunks
HQ = H // 4  # 256, one h-quarter of W per DMA

F32 = mybir.dt.float32
BF16 = mybir.dt.bfloat16
AF = mybir.ActivationFunctionType
OP = mybir.AluOpType

_CACHE = {}


def _mingru_tile(tc, out, x, h0, wzT, bz, whT, bh):
    nc = tc.nc

    with ExitStack() as ctx:
        consts = ctx.enter_context(tc.tile_pool(name="consts", bufs=1))

        id_bf = consts.tile([P, P], BF16)
        make_identity(nc, id_bf)

        # Small strided gathers on the otherwise idle SWDGE queue.
        bz_sb = consts.tile([P, HC], F32)
        nc.gpsimd.dma_start(out=bz_sb, in_=bz.rearrange("(c p) -> p c", p=P))
        bh_sb = consts.tile([P, HC], F32)
        nc.gpsimd.dma_start(out=bh_sb, in_=bh.rearrange("(c p) -> p c", p=P))
        hp_sb = consts.tile([P, B_LOC * HC], F32)
        nc.gpsimd.dma_start(out=hp_sb, in_=h0.rearrange("b (c p) -> p (b c)", p=P))
        nbz_sb = consts.tile([P, HC], F32)
        nc.vector.tensor_scalar_mul(nbz_sb, bz_sb, -1.0)

        xt_p = ctx.enter_context(tc.tile_pool(name="xt", bufs=2))
        azb_p = ctx.enter_context(tc.tile_pool(name="azb", bufs=2))
        scan_p = ctx.enter_context(tc.tile_pool(name="scan", bufs=2))
        xnat_p = ctx.enter_context(tc.tile_pool(name="xnat", bufs=1))

        def step_bt(s):
            return s // NTC, s % NTC

        # NOTE: both the SBUF tile creation order AND the DMA issue order
        # below are load-bearing. Reordering the consts-pool tiles was
        # measured to slow every PE instruction ~20%, and batching the W
        # quarter issues was measured to block the issuing engine on
        # HWDGE queue-capacity semaphores for >10 us, starving the
        # prologue copies emitted behind them. Keep x row-blocks first
        # (alternating queues), then W quarters interleaved z->SP/h->ACT.

        # Step 0's x, natural layout, plain loads split across both HWDGE
        # queues (each queue's small in-flight window paces its own W half).
        xn0 = []
        for j in range(TSUB):
            t_ = xnat_p.tile([P, D], BF16, tag=f"xn{j}", name=f"xn0_{j}")
            eng = nc.scalar if j % 2 else nc.sync
            eng.dma_start(out=t_, in_=x[0, ds(j * P, P), :])
            xn0.append(t_)

        # W^T arrives pre-transposed [D, H] bf16 from the host. One strided
        # DMA per h-quarter (512B row chunks); W_z quarters on the SP queue,
        # W_h on ACT:
        #   wt[wn][r][p, dc*HQ + h'] = W^T[dc*128 + p, r*HQ + h']
        # lhsT block (hc,dc) = wt[wn][hc//2][:, dc*HQ + (hc%2)*128 ...].
        wt = {"z": [], "h": []}

        def w_load(wn, r):
            w_ap, w_eng = (wzT, nc.sync) if wn == "z" else (whT, nc.scalar)
            w_sb = consts.tile([P, DC * HQ], BF16, name=f"wt_{wn}{r}")
            w_eng.dma_start(
                out=w_sb.rearrange("p (dc h) -> p dc h", h=HQ),
                in_=w_ap[:, ds(r * HQ, HQ)].rearrange("(dc p) h -> p dc h", p=P),
            )
            wt[wn].append(w_sb)

        for r in range(4):
            w_load("z", r)
            w_load("h", r)

        xts = {}

        def t_x(s):  # crossbar transpose, issued a full step ahead of use
            b, tci = step_bt(s)
            tiles = []
            for dc in range(DC):
                t_ = xt_p.tile([P, TC], BF16, tag=f"xt{dc}", name=f"xt_{s}_{dc}")
                nc.sync.dma_start_transpose(t_, x[b, ds(tci * TC, TC), ts(dc, P)])
                tiles.append(t_)
            xts[s] = tiles

        t_x(1)

        # Prologue PE work: HAM warmup junk, then step 0's x^T on the PE
        # (transpose to PSUM j-outer in two 4-dc waves so work starts as
        # each xn row-block lands; ACT/DVE alternate the PSUM->SBUF copies).
        xts[0] = []
        with tc.tile_pool(name="warm", bufs=1, space="PSUM") as warm_p, \
             tc.tile_pool(name="pxt", bufs=1, space="PSUM") as pxt_p, \
             tc.tile_pool(name="wdram", bufs=1, space="DRAM") as wdram_p:
            junk_ps = warm_p.tile([P, P], F32, name="junk_ps")
            NWARM = 30
            for i in range(NWARM):
                nc.tensor.matmul(
                    junk_ps, id_bf, id_bf, start=(i == 0), stop=(i == NWARM - 1)
                )
            junk_sb = consts.tile([P, P], F32, name="junk_sb")
            nc.vector.tensor_copy(junk_sb, junk_ps)
            junk_dr = wdram_p.tile([P, P], F32, name="junk_dr")
            nc.sync.dma_start(out=junk_dr, in_=junk_sb)

            for wave in range(2):
                pxts = [
                    pxt_p.tile([P, TC], BF16, tag=f"pxt{k}", name=f"pxt_{wave}_{k}")
                    for k in range(4)
                ]
                for j in range(TSUB):
                    for k in range(4):
                        dc = wave * 4 + k
                        nc.tensor.transpose(
                            pxts[k][:, ts(j, P)], xn0[j][:, ts(dc, P)], id_bf
                        )
                for k in range(4):
                    dc = wave * 4 + k
                    xt_sb = xt_p.tile([P, TC], BF16, tag=f"xt{dc}", name=f"xt_0_{dc}")
                    eng = nc.scalar if k % 2 else nc.vector
                    if k % 2:
                        nc.scalar.copy(xt_sb, pxts[k])
                    else:
                        nc.vector.tensor_copy(xt_sb, pxts[k])
                    xts[0].append(xt_sb)

        # PSUM: 4 z + 4 h GEMM banks (prologue banks are re-used once the
        # ACT/DVE copies above have drained — before the first GEMM needs
        # them).
        pz_p = ctx.enter_context(tc.tile_pool(name="pz", bufs=4, space="PSUM"))
        ph_p = ctx.enter_context(tc.tile_pool(name="ph", bufs=4, space="PSUM"))

        scans = {}

        def gemm(s, hc, wn):
            pool = pz_p if wn == "z" else ph_p
            psum = pool.tile([P, TC], F32, tag="p" + wn, name=f"ps{wn}_{s}_{hc}")
            xt = xts[s]
            w_sb = wt[wn][hc // 2]
            for dc in range(DC):
                nc.tensor.matmul(
                    psum,
                    w_sb[:, ds(dc * HQ + (hc % 2) * P, P)],
                    xt[dc],
                    start=(dc == 0),
                    stop=(dc == DC - 1),
                )
            return psum

        def post(s, hc, psum_z, psum_h):
            b, tci = step_bt(s)
            a_sb = azb_p.tile([P, TC], BF16, tag="a", name=f"a_{s}_{hc}")
            nc.scalar.activation(
                a_sb, psum_z, AF.Sigmoid, bias=nbz_sb[:, hc : hc + 1], scale=-1.0
            )
            z_sb = azb_p.tile([P, TC], F32, tag="z", name=f"z_{s}_{hc}")
            nc.scalar.activation(
                z_sb, psum_z, AF.Sigmoid, bias=bz_sb[:, hc : hc + 1], scale=1.0
            )
            bsc = azb_p.tile([P, TC], BF16, tag="b", name=f"b_{s}_{hc}")
            nc.vector.scalar_tensor_tensor(
                bsc, psum_h, bh_sb[:, hc : hc + 1], z_sb, op0=OP.add, op1=OP.mult
            )
            # bf16 scan output: the scan accumulator is fp32 in HW
            # regardless of out dtype, so only stored values round; bf16
            # halves the store bytes.
            sc = scan_p.tile([P, TC], BF16, tag=f"sc{hc}", name=f"sc_{s}_{hc}")
            if tci == 0:
                init = hp_sb[:, b * HC + hc : b * HC + hc + 1]
            else:
                init = scans[s - 1][hc][:, TC - 1 : TC]
            nc.vector.tensor_tensor_scan(sc, a_sb, bsc, init, op0=OP.mult, op1=OP.add)
            scans.setdefault(s, [None] * HC)[hc] = sc
            # Store the scan tile directly in [H, T] layout on the SP
            # HWDGE queue (host transposes). The store's scan-semaphore
            # wait must NOT sit on the ACT engine: it would serialize the
            # sigmoids (which drain the GEMM PSUM banks) behind the DVE
            # scan chain and stall the PE. The SP engine only issues the
            # next-next step's crossbar loads after these, which is safe.
            # Last step only: ACT has no future sigmoids to poison, so
            # split the final stores across both queues to drain faster.
            eng = nc.scalar if (s == NSTEP - 1 and hc % 2) else nc.sync
            eng.dma_start(
                out=out[b, ds(hc * P, P), ds(tci * TC, TC)], in_=sc
            )

        # --- steps -------------------------------------------------------
        for s in range(NSTEP):
            if s == 0:
                # GEMMs in z/h quarter-pairs so they track the W quarter
                # DMAs landing on the two queues.
                for r in range(4):
                    pzs = [gemm(0, 2 * r + i, "z") for i in range(2)]
                    phs = [gemm(0, 2 * r + i, "h") for i in range(2)]
                    for i in range(2):
                        post(0, 2 * r + i, pzs[i], phs[i])
            else:
                if s + 1 < NSTEP:
                    t_x(s + 1)
                for hc in range(HC):
                    psum_z = gemm(s, hc, "z")
                    psum_h = gemm(s, hc, "h")
                    post(s, hc, psum_z, psum_h)
                if s - 2 in scans:
                    del scans[s - 2]


def build():
    if "nc" in _CACHE:
        return _CACHE["nc"]
    nc = bacc.Bacc(
        "TRN2", target_bir_lowering=False, debug=False, num_devices=NCORES
    )
    x = nc.dram_tensor("x", [B_LOC, T, D], BF16, kind="ExternalInput").ap()
    h0 = nc.dram_tensor("h0", [B_LOC, H], F32, kind="ExternalInput").ap()
    wzT = nc.dram_tensor("wzT", [D, H], BF16, kind="ExternalInput").ap()
    bz = nc.dram_tensor("bz", [H], F32, kind="ExternalInput").ap()
    whT = nc.dram_tensor("whT", [D, H], BF16, kind="ExternalInput").ap()
    bh = nc.dram_tensor("bh", [H], F32, kind="ExternalInput").ap()
    out = nc.dram_tensor("out", [B_LOC, H, T], BF16, kind="ExternalOutput").ap()
    with tile.TileContext(nc) as tctx:
        _mingru_tile(tctx, out, x, h0, wzT, bz, whT, bh)
    nc.compile()
    _CACHE["nc"] = nc
    return nc


def make_in_maps(x, h_prev, W_z, b_z, W_h, b_h):
    x = np.asarray(x, dtype=np.float32).astype(ml_dtypes.bfloat16)
    h_prev = np.ascontiguousarray(np.asarray(h_prev, dtype=np.float32))
    wzT = np.asarray(W_z, dtype=np.float32).T.astype(ml_dtypes.bfloat16)
    whT = np.asarray(W_h, dtype=np.float32).T.astype(ml_dtypes.bfloat16)
    b_z = np.ascontiguousarray(np.asarray(b_z, dtype=np.float32))
    b_h = np.ascontiguousarray(np.asarray(b_h, dtype=np.float32))
    in_maps = []
    for c in range(NCORES):
        sl = slice(c * B_LOC, (c + 1) * B_LOC)
        in_maps.append(
            {
                "x": np.ascontiguousarray(x[sl]),
                "h0": h_prev[sl],
                "wzT": wzT,
                "bz": b_z,
                "whT": whT,
                "bh": b_h,
            }
        )
    return in_maps


def kernel(x, h_prev, W_z, b_z, W_h, b_h, trace=False):
    nc = build()
    in_maps = make_in_maps(x, h_prev, W_z, b_z, W_h, b_h)
    res = run_bass_kernel_spmd(
        nc, in_maps, core_ids=list(range(NCORES)), trace=trace
    )
    # Device output is [B_loc, H, T] bf16; transpose to (B, T, H) f32 here.
    out = np.concatenate(
        [
            np.asarray(r["out"]).astype(np.float32).transpose(0, 2, 1)
            for r in res.results
        ],
        axis=0,
    )
    if trace:
        _CACHE["last_results"] = res
    return out


# revision 13
# speedup vs baseline: 1.1884x; 1.0041x over previous
"""MinGRU cell on 8 Trainium2 NeuronCores (Bass/Tile).

Math (per batch b, hidden h):
    gz = x @ W_z^T ; gh = x @ W_h^T                 (two GEMMs, K=D=1024)
    z  = sigmoid(gz + b_z)
    h_t = (1 - z_t) * h_{t-1} + z_t * (gh_t + b_h)  (affine scan over T)

Distribution: data-parallel over batch B=16 -> 2 batches per core, weights
replicated; no cross-core communication.

Per-core steady state: the PE streams the GEMMs (128 N=512 bf16 matmuls
per 512-token step, ~218 ns each = the bf16 roofline) with no other PE
work: the scan output is stored to DRAM in [H, T] layout straight from
the DVE scan tiles and transposed to [T, H] on the host, so the PE's
out-transposes (and their ACT/DVE copies and the 16 us drain tail) are
gone entirely. x^T tiles for steps 1..7 are produced by the DMA crossbar
(dma_start_transpose) straight from (host-precast bf16) DRAM, issued a
full step ahead — the crossbar's completion semaphore has been observed
to lead its data on profiled runs, so every crossbar transpose here has
~10+ us between data landing and first consumer. Step 0's x^T is built
on the PE instead (plain loads + tensor-engine transposes, j-outer so
transposes start as soon as each x row-block lands). W arrives
pre-transposed bf16 from the host (weight pre-packing) in four half-H
strided DMAs per W; step-0 GEMMs are ordered in z/h quarter-pairs so
they track the W quarters as they land on the two HWDGE queues (W_z on
SP, W_h on ACT). Bias/h0 gathers go to the GpSimd SWDGE queue. ACT runs
the two sigmoids (z and 1-z), DVE the (gh+b_h)*z fuse and the affine
scan. Output is written bf16 and upcast to f32 on the host (it was
computed in bf16 either way).
"""

import sys

sys.path.insert(0, "/opt/trn_rl_repo")

from contextlib import ExitStack

import numpy as np
import ml_dtypes

import concourse.bass as bass
import concourse.mybir as mybir
import concourse.tile as tile
from concourse import bacc
from concourse.bass import ts, ds
from concourse.bass_utils import run_bass_kernel_spmd
from concourse.masks import make_identity

B, T, D, H = 16, 2048, 1024, 1024
NCORES = 8
B_LOC = B // NCORES  # 2
P = 128
TC = 512  # tokens per step
NSTEP = B_LOC * T // TC  # 8
NTC = T // TC  # 4 steps per batch
TSUB = TC // P  # 4
DC = D // P  # 8 contraction chunks
HC = H // P  # 8 hidden chunks
HQ = H // 4  # 256, one h-quarter of W per DMA

F32 = mybir.dt.float32
BF16 = mybir.dt.bfloat16
AF = mybir.ActivationFunctionType
OP = mybir.AluOpType

_CACHE = {}


def _mingru_tile(tc, out, x, h0, wzT, bz, whT, bh):
    nc = tc.nc

    with ExitStack() as ctx:
        consts = ctx.enter_context(tc.tile_pool(name="consts", bufs=1))

        id_bf = consts.tile([P, P], BF16)
        make_identity(nc, id_bf)

        # Small strided gathers on the otherwise idle SWDGE queue.
        bz_sb = consts.tile([P, HC], F32)
        nc.gpsimd.dma_start(out=bz_sb, in_=bz.rearrange("(c p) -> p c", p=P))
        bh_sb = consts.tile([P, HC], F32)
        nc.gpsimd.dma_start(out=bh_sb, in_=bh.rearrange("(c p) -> p c", p=P))
        hp_sb = consts.tile([P, B_LOC * HC], F32)
        nc.gpsimd.dma_start(out=hp_sb, in_=h0.rearrange("b (c p) -> p (b c)", p=P))
        nbz_sb = consts.tile([P, HC], F32)
        nc.vector.tensor_scalar_mul(nbz_sb, bz_sb, -1.0)

        xt_p = ctx.enter_context(tc.tile_pool(name="xt", bufs=2))
        azb_p = ctx.enter_context(tc.tile_pool(name="azb", bufs=2))
        scan_p = ctx.enter_context(tc.tile_pool(name="scan", bufs=2))
        xnat_p = ctx.enter_context(tc.tile_pool(name="xnat", bufs=1))

        def step_bt(s):
            return s // NTC, s % NTC

        # NOTE: both the SBUF tile creation order AND the DMA issue order
        # below are load-bearing. Reordering the consts-pool tiles was
        # measured to slow every PE instruction ~20%, and batching >6 big
        # DMA issues on one engine was measured to block that engine on
        # HWDGE queue-capacity semaphores for >10 us, starving the work
        # emitted behind them. Tiles are created in the measured-good
        # order; DMA issue is scheduled separately below.

        # Step 0's x, natural layout, as two 2-row-block DMAs.
        xn01 = xnat_p.tile([P, 2, D], BF16, tag="xn01", name="xn01")
        xn23 = xnat_p.tile([P, 2, D], BF16, tag="xn23", name="xn23")

        def xnt(j):
            return (xn01 if j < 2 else xn23)[:, j % 2]

        # W^T arrives pre-transposed [D, H] bf16 from the host. One strided
        # DMA per h-quarter (512B row chunks):
        #   wt[wn][r][p, dc*HQ + h'] = W^T[dc*128 + p, r*HQ + h']
        # lhsT block (hc,dc) = wt[wn][hc//2][:, dc*HQ + (hc%2)*128 ...].
        wt = {"z": [], "h": []}
        for r in range(4):
            for wn in ("z", "h"):
                wt[wn].append(consts.tile([P, DC * HQ], BF16, name=f"wt_{wn}{r}"))

        def w_issue(wn, r, eng):
            w_ap = wzT if wn == "z" else whT
            eng.dma_start(
                out=wt[wn][r].rearrange("p (dc h) -> p dc h", h=HQ),
                in_=w_ap[:, ds(r * HQ, HQ)].rearrange("(dc p) h -> p dc h", p=P),
            )

        # Queue plan. The SP engine runs the framework's semaphore-init
        # preamble and starts its queue ~5 us after ACT's, so the
        # critical z quarter-0 leads the ACT queue; x's two row-block
        # pairs (which gate the PE transpose prologue) go next on each
        # queue; the remaining quarters are placed so each lands a few
        # microseconds before the step-0 quarter-pair GEMMs consume it,
        # with at most 4 big issues on ACT and 6 on SP.
        w_issue("z", 0, nc.scalar)
        nc.scalar.dma_start(
            out=xn01, in_=x[0, ds(0, 2 * P), :].rearrange("(j p) d -> p j d", p=P)
        )
        nc.sync.dma_start(
            out=xn23, in_=x[0, ds(2 * P, 2 * P), :].rearrange("(j p) d -> p j d", p=P)
        )
        w_issue("h", 0, nc.scalar)
        w_issue("h", 1, nc.scalar)
        w_issue("z", 1, nc.sync)
        w_issue("z", 2, nc.sync)
        w_issue("z", 3, nc.sync)
        w_issue("h", 2, nc.sync)
        w_issue("h", 3, nc.sync)

        xts = {}

        def t_x(s):  # crossbar transpose, issued a full step ahead of use
            b, tci = step_bt(s)
            tiles = []
            for dc in range(DC):
                t_ = xt_p.tile([P, TC], BF16, tag=f"xt{dc}", name=f"xt_{s}_{dc}")
                nc.sync.dma_start_transpose(t_, x[b, ds(tci * TC, TC), ts(dc, P)])
                tiles.append(t_)
            xts[s] = tiles

        t_x(1)

        # Prologue PE work: HAM warmup junk, then step 0's x^T on the PE
        # (transpose to PSUM j-outer in two 4-dc waves so work starts as
        # each xn row-block lands; ACT/DVE alternate the PSUM->SBUF copies).
        xts[0] = []
        with tc.tile_pool(name="warm", bufs=1, space="PSUM") as warm_p, \
             tc.tile_pool(name="pxt", bufs=1, space="PSUM") as pxt_p, \
             tc.tile_pool(name="wdram", bufs=1, space="DRAM") as wdram_p:
            junk_ps = warm_p.tile([P, P], F32, name="junk_ps")
            NWARM = 30
            for i in range(NWARM):
                nc.tensor.matmul(
                    junk_ps, id_bf, id_bf, start=(i == 0), stop=(i == NWARM - 1)
                )
            junk_sb = consts.tile([P, P], F32, name="junk_sb")
            nc.vector.tensor_copy(junk_sb, junk_ps)
            junk_dr = wdram_p.tile([P, P], F32, name="junk_dr")
            nc.sync.dma_start(out=junk_dr, in_=junk_sb)

            for wave in range(2):
                pxts = [
                    pxt_p.tile([P, TC], BF16, tag=f"pxt{k}", name=f"pxt_{wave}_{k}")
                    for k in range(4)
                ]
                for j in range(TSUB):
                    for k in range(4):
                        dc = wave * 4 + k
                        nc.tensor.transpose(
                            pxts[k][:, ts(j, P)], xnt(j)[:, ts(dc, P)], id_bf
                        )
                for k in range(4):
                    dc = wave * 4 + k
                    xt_sb = xt_p.tile([P, TC], BF16, tag=f"xt{dc}", name=f"xt_0_{dc}")
                    eng = nc.scalar if k % 2 else nc.vector
                    if k % 2:
                        nc.scalar.copy(xt_sb, pxts[k])
                    else:
                        nc.vector.tensor_copy(xt_sb, pxts[k])
                    xts[0].append(xt_sb)

        # PSUM: 4 z + 4 h GEMM banks (prologue banks are re-used once the
        # ACT/DVE copies above have drained — before the first GEMM needs
        # them).
        pz_p = ctx.enter_context(tc.tile_pool(name="pz", bufs=4, space="PSUM"))
        ph_p = ctx.enter_context(tc.tile_pool(name="ph", bufs=4, space="PSUM"))

        scans = {}

        def gemm(s, hc, wn):
            pool = pz_p if wn == "z" else ph_p
            psum = pool.tile([P, TC], F32, tag="p" + wn, name=f"ps{wn}_{s}_{hc}")
            xt = xts[s]
            w_sb = wt[wn][hc // 2]
            for dc in range(DC):
                nc.tensor.matmul(
                    psum,
                    w_sb[:, ds(dc * HQ + (hc % 2) * P, P)],
                    xt[dc],
                    start=(dc == 0),
                    stop=(dc == DC - 1),
                )
            return psum

        def post(s, hc, psum_z, psum_h):
            b, tci = step_bt(s)
            a_sb = azb_p.tile([P, TC], BF16, tag="a", name=f"a_{s}_{hc}")
            nc.scalar.activation(
                a_sb, psum_z, AF.Sigmoid, bias=nbz_sb[:, hc : hc + 1], scale=-1.0
            )
            z_sb = azb_p.tile([P, TC], F32, tag="z", name=f"z_{s}_{hc}")
            nc.scalar.activation(
                z_sb, psum_z, AF.Sigmoid, bias=bz_sb[:, hc : hc + 1], scale=1.0
            )
            bsc = azb_p.tile([P, TC], BF16, tag="b", name=f"b_{s}_{hc}")
            nc.vector.scalar_tensor_tensor(
                bsc, psum_h, bh_sb[:, hc : hc + 1], z_sb, op0=OP.add, op1=OP.mult
            )
            # bf16 scan output: the scan accumulator is fp32 in HW
            # regardless of out dtype, so only stored values round; bf16
            # halves the store bytes.
            sc = scan_p.tile([P, TC], BF16, tag=f"sc{hc}", name=f"sc_{s}_{hc}")
            if tci == 0:
                init = hp_sb[:, b * HC + hc : b * HC + hc + 1]
            else:
                init = scans[s - 1][hc][:, TC - 1 : TC]
            nc.vector.tensor_tensor_scan(sc, a_sb, bsc, init, op0=OP.mult, op1=OP.add)
            scans.setdefault(s, [None] * HC)[hc] = sc
            # Store the scan tile directly in [H, T] layout on the SP
            # HWDGE queue (host transposes). The store's scan-semaphore
            # wait must NOT sit on the ACT engine: it would serialize the
            # sigmoids (which drain the GEMM PSUM banks) behind the DVE
            # scan chain and stall the PE. The SP engine only issues the
            # next-next step's crossbar loads after these, which is safe.
            nc.sync.dma_start(
                out=out[b, ds(hc * P, P), ds(tci * TC, TC)], in_=sc
            )

        # --- steps -------------------------------------------------------
        for s in range(NSTEP):
            if s == 0:
                # GEMMs in z/h quarter-pairs so they track the W quarter
                # DMAs landing on the two queues.
                for r in range(4):
                    pzs = [gemm(0, 2 * r + i, "z") for i in range(2)]
                    phs = [gemm(0, 2 * r + i, "h") for i in range(2)]
                    for i in range(2):
                        post(0, 2 * r + i, pzs[i], phs[i])
            else:
                if s + 1 < NSTEP:
                    t_x(s + 1)
                for hc in range(HC):
                    psum_z = gemm(s, hc, "z")
                    psum_h = gemm(s, hc, "h")
                    post(s, hc, psum_z, psum_h)
                if s - 2 in scans:
                    del scans[s - 2]


def build():
    if "nc" in _CACHE:
        return _CACHE["nc"]
    nc = bacc.Bacc(
        "TRN2", target_bir_lowering=False, debug=False, num_devices=NCORES
    )
    x = nc.dram_tensor("x", [B_LOC, T, D], BF16, kind="ExternalInput").ap()
    h0 = nc.dram_tensor("h0", [B_LOC, H], F32, kind="ExternalInput").ap()
    wzT = nc.dram_tensor("wzT", [D, H], BF16, kind="ExternalInput").ap()
    bz = nc.dram_tensor("bz", [H], F32, kind="ExternalInput").ap()
    whT = nc.dram_tensor("whT", [D, H], BF16, kind="ExternalInput").ap()
    bh = nc.dram_tensor("bh", [H], F32, kind="ExternalInput").ap()
    out = nc.dram_tensor("out", [B_LOC, H, T], BF16, kind="ExternalOutput").ap()
    with tile.TileContext(nc) as tctx:
        _mingru_tile(tctx, out, x, h0, wzT, bz, whT, bh)
    nc.compile()
    _CACHE["nc"] = nc
    return nc


def make_in_maps(x, h_prev, W_z, b_z, W_h, b_h):
    x = np.asarray(x, dtype=np.float32).astype(ml_dtypes.bfloat16)
    h_prev = np.ascontiguousarray(np.asarray(h_prev, dtype=np.float32))
    wzT = np.asarray(W_z, dtype=np.float32).T.astype(ml_dtypes.bfloat16)
    whT = np.asarray(W_h, dtype=np.float32).T.astype(ml_dtypes.bfloat16)
    b_z = np.ascontiguousarray(np.asarray(b_z, dtype=np.float32))
    b_h = np.ascontiguousarray(np.asarray(b_h, dtype=np.float32))
    in_maps = []
    for c in range(NCORES):
        sl = slice(c * B_LOC, (c + 1) * B_LOC)
        in_maps.append(
            {
                "x": np.ascontiguousarray(x[sl]),
                "h0": h_prev[sl],
                "wzT": wzT,
                "bz": b_z,
                "whT": whT,
                "bh": b_h,
            }
        )
    return in_maps


def kernel(x, h_prev, W_z, b_z, W_h, b_h, trace=False):
    nc = build()
    in_maps = make_in_maps(x, h_prev, W_z, b_z, W_h, b_h)
    res = run_bass_kernel_spmd(
        nc, in_maps, core_ids=list(range(NCORES)), trace=trace
    )
    # Device output is [B_loc, H, T] bf16; transpose to (B, T, H) f32 here.
    out = np.concatenate(
        [
            np.asarray(r["out"]).astype(np.float32).transpose(0, 2, 1)
            for r in res.results
        ],
        axis=0,
    )
    if trace:
        _CACHE["last_results"] = res
    return out


# revision 15
# speedup vs baseline: 1.1909x; 1.0021x over previous
"""MinGRU cell on 8 Trainium2 NeuronCores (Bass/Tile).

Math (per batch b, hidden h):
    gz = x @ W_z^T ; gh = x @ W_h^T                 (two GEMMs, K=D=1024)
    z  = sigmoid(gz + b_z)
    h_t = (1 - z_t) * h_{t-1} + z_t * (gh_t + b_h)  (affine scan over T)

Distribution: data-parallel over batch B=16 -> 2 batches per core, weights
replicated; no cross-core communication.

Per-core steady state: the PE streams the GEMMs (128 N=512 bf16 matmuls
per 512-token step, ~218 ns each = the bf16 roofline) with no other PE
work: the scan output is stored to DRAM in [H, T] layout straight from
the DVE scan tiles and transposed to [T, H] on the host, so the PE's
out-transposes (and their ACT/DVE copies and the 16 us drain tail) are
gone entirely. x^T tiles for steps 1..7 are produced by the DMA crossbar
(dma_start_transpose) straight from (host-precast bf16) DRAM, issued a
full step ahead — the crossbar's completion semaphore has been observed
to lead its data on profiled runs, so every crossbar transpose here has
~10+ us between data landing and first consumer. Step 0's x^T is built
on the PE instead (plain loads + tensor-engine transposes, j-outer so
transposes start as soon as each x row-block lands). W arrives
pre-transposed bf16 from the host (weight pre-packing) in four half-H
strided DMAs per W; step-0 GEMMs are ordered in z/h quarter-pairs so
they track the W quarters as they land on the two HWDGE queues (W_z on
SP, W_h on ACT). Bias/h0 gathers go to the GpSimd SWDGE queue. ACT runs
the two sigmoids (z and 1-z), DVE the (gh+b_h)*z fuse and the affine
scan. Output is written bf16 and upcast to f32 on the host (it was
computed in bf16 either way).
"""

import sys

sys.path.insert(0, "/opt/trn_rl_repo")

from contextlib import ExitStack

import numpy as np
import ml_dtypes

import concourse.bass as bass
import concourse.mybir as mybir
import concourse.tile as tile
from concourse import bacc
from concourse.bass import ts, ds
from concourse.bass_utils import run_bass_kernel_spmd
from concourse.masks import make_identity

B, T, D, H = 16, 2048, 1024, 1024
NCORES = 8
B_LOC = B // NCORES  # 2
P = 128
TC = 512  # tokens per step
NSTEP = B_LOC * T // TC  # 8
NTC = T // TC  # 4 steps per batch
TSUB = TC // P  # 4
DC = D // P  # 8 contraction chunks
HC = H // P  # 8 hidden chunks
HQ = H // 4  # 256, one h-quarter of W per DMA

F32 = mybir.dt.float32
BF16 = mybir.dt.bfloat16
AF = mybir.ActivationFunctionType
OP = mybir.AluOpType

_CACHE = {}


def _mingru_tile(tc, out, x, h0, wzT, bz, whT, bh):
    nc = tc.nc

    with ExitStack() as ctx:
        consts = ctx.enter_context(tc.tile_pool(name="consts", bufs=1))

        id_bf = consts.tile([P, P], BF16)
        make_identity(nc, id_bf)

        # Small strided gathers on the otherwise idle SWDGE queue.
        bz_sb = consts.tile([P, HC], F32)
        nc.gpsimd.dma_start(out=bz_sb, in_=bz.rearrange("(c p) -> p c", p=P))
        bh_sb = consts.tile([P, HC], F32)
        nc.gpsimd.dma_start(out=bh_sb, in_=bh.rearrange("(c p) -> p c", p=P))
        hp_sb = consts.tile([P, B_LOC * HC], F32)
        nc.gpsimd.dma_start(out=hp_sb, in_=h0.rearrange("b (c p) -> p (b c)", p=P))
        nbz_sb = consts.tile([P, HC], F32)
        nc.vector.tensor_scalar_mul(nbz_sb, bz_sb, -1.0)

        xt_p = ctx.enter_context(tc.tile_pool(name="xt", bufs=2))
        azb_p = ctx.enter_context(tc.tile_pool(name="azb", bufs=2))
        scan_p = ctx.enter_context(tc.tile_pool(name="scan", bufs=2))
        xnat_p = ctx.enter_context(tc.tile_pool(name="xnat", bufs=1))

        def step_bt(s):
            return s // NTC, s % NTC

        # NOTE: both the SBUF tile creation order AND the DMA issue order
        # below are load-bearing. Reordering the consts-pool tiles was
        # measured to slow every PE instruction ~20%, and batching >6 big
        # DMA issues on one engine was measured to block that engine on
        # HWDGE queue-capacity semaphores for >10 us, starving the work
        # emitted behind them. Tiles are created in the measured-good
        # order; DMA issue is scheduled separately below.

        # Step 0's x, natural layout, as two 2-row-block DMAs.
        xn01 = xnat_p.tile([P, 2, D], BF16, tag="xn01", name="xn01")
        xn23 = xnat_p.tile([P, 2, D], BF16, tag="xn23", name="xn23")

        def xnt(j):
            return (xn01 if j < 2 else xn23)[:, j % 2]

        # W^T arrives pre-transposed [D, H] bf16 from the host. One strided
        # DMA per h-quarter (512B row chunks):
        #   wt[wn][r][p, dc*HQ + h'] = W^T[dc*128 + p, r*HQ + h']
        # lhsT block (hc,dc) = wt[wn][hc//2][:, dc*HQ + (hc%2)*128 ...].
        wt = {"z": [], "h": []}
        for r in range(4):
            for wn in ("z", "h"):
                wt[wn].append(consts.tile([P, DC * HQ], BF16, name=f"wt_{wn}{r}"))

        def w_issue(wn, r, eng):
            w_ap = wzT if wn == "z" else whT
            eng.dma_start(
                out=wt[wn][r].rearrange("p (dc h) -> p dc h", h=HQ),
                in_=w_ap[:, ds(r * HQ, HQ)].rearrange("(dc p) h -> p dc h", p=P),
            )

        # Queue plan. The SP engine runs the framework's semaphore-init
        # preamble and starts its queue ~5 us after ACT's, so the
        # critical z quarter-0 leads the ACT queue; x's two row-block
        # pairs (which gate the PE transpose prologue) go next on each
        # queue; the remaining quarters are placed so each lands a few
        # microseconds before the step-0 quarter-pair GEMMs consume it,
        # with at most 4 big issues on ACT and 6 on SP.
        nc.sync.dma_start(
            out=xn01, in_=x[0, ds(0, 2 * P), :].rearrange("(j p) d -> p j d", p=P)
        )
        nc.scalar.dma_start(
            out=xn23, in_=x[0, ds(2 * P, 2 * P), :].rearrange("(j p) d -> p j d", p=P)
        )
        for r in range(4):
            w_issue("z", r, nc.sync)
            w_issue("h", r, nc.scalar)

        xts = {}

        def t_x(s):  # crossbar transpose, issued a full step ahead of use
            b, tci = step_bt(s)
            tiles = []
            for dc in range(DC):
                t_ = xt_p.tile([P, TC], BF16, tag=f"xt{dc}", name=f"xt_{s}_{dc}")
                nc.sync.dma_start_transpose(t_, x[b, ds(tci * TC, TC), ts(dc, P)])
                tiles.append(t_)
            xts[s] = tiles

        t_x(1)

        # Prologue PE work: HAM warmup junk, then step 0's x^T on the PE
        # (transpose to PSUM j-outer in two 4-dc waves so work starts as
        # each xn row-block lands; ACT/DVE alternate the PSUM->SBUF copies).
        xts[0] = []
        with tc.tile_pool(name="warm", bufs=1, space="PSUM") as warm_p, \
             tc.tile_pool(name="pxt", bufs=1, space="PSUM") as pxt_p, \
             tc.tile_pool(name="wdram", bufs=1, space="DRAM") as wdram_p:
            # Long sustained warmup: the HAM power-ramp limiter clamps PE
            # utilization to 0.5 for ~10 us starting when it first sees
            # sustained matmul activity. 30 tiny matmuls were too little
            # to trigger it, so the clamp fired on the first REAL GEMMs
            # (~16-26 us). ~100 back-to-back matmuls (~5.5 us of PE) pull
            # the ramp window into the DMA-bound startup idle instead.
            junk_ps = warm_p.tile([P, P], F32, name="junk_ps")
            NWARM = 100
            for i in range(NWARM):
                nc.tensor.matmul(
                    junk_ps, id_bf, id_bf, start=(i == 0), stop=(i == NWARM - 1)
                )
            junk_sb = consts.tile([P, P], F32, name="junk_sb")
            nc.vector.tensor_copy(junk_sb, junk_ps)
            junk_dr = wdram_p.tile([P, P], F32, name="junk_dr")
            nc.sync.dma_start(out=junk_dr, in_=junk_sb)

            for wave in range(2):
                pxts = [
                    pxt_p.tile([P, TC], BF16, tag=f"pxt{k}", name=f"pxt_{wave}_{k}")
                    for k in range(4)
                ]
                for j in range(TSUB):
                    for k in range(4):
                        dc = wave * 4 + k
                        nc.tensor.transpose(
                            pxts[k][:, ts(j, P)], xnt(j)[:, ts(dc, P)], id_bf
                        )
                for k in range(4):
                    dc = wave * 4 + k
                    xt_sb = xt_p.tile([P, TC], BF16, tag=f"xt{dc}", name=f"xt_0_{dc}")
                    eng = nc.scalar if k % 2 else nc.vector
                    if k % 2:
                        nc.scalar.copy(xt_sb, pxts[k])
                    else:
                        nc.vector.tensor_copy(xt_sb, pxts[k])
                    xts[0].append(xt_sb)

        # PSUM: 4 z + 4 h GEMM banks (prologue banks are re-used once the
        # ACT/DVE copies above have drained — before the first GEMM needs
        # them).
        pz_p = ctx.enter_context(tc.tile_pool(name="pz", bufs=4, space="PSUM"))
        ph_p = ctx.enter_context(tc.tile_pool(name="ph", bufs=4, space="PSUM"))

        scans = {}

        def gemm(s, hc, wn):
            pool = pz_p if wn == "z" else ph_p
            psum = pool.tile([P, TC], F32, tag="p" + wn, name=f"ps{wn}_{s}_{hc}")
            xt = xts[s]
            w_sb = wt[wn][hc // 2]
            for dc in range(DC):
                nc.tensor.matmul(
                    psum,
                    w_sb[:, ds(dc * HQ + (hc % 2) * P, P)],
                    xt[dc],
                    start=(dc == 0),
                    stop=(dc == DC - 1),
                )
            return psum

        def post(s, hc, psum_z, psum_h):
            b, tci = step_bt(s)
            a_sb = azb_p.tile([P, TC], BF16, tag="a", name=f"a_{s}_{hc}")
            nc.scalar.activation(
                a_sb, psum_z, AF.Sigmoid, bias=nbz_sb[:, hc : hc + 1], scale=-1.0
            )
            z_sb = azb_p.tile([P, TC], F32, tag="z", name=f"z_{s}_{hc}")
            nc.scalar.activation(
                z_sb, psum_z, AF.Sigmoid, bias=bz_sb[:, hc : hc + 1], scale=1.0
            )
            bsc = azb_p.tile([P, TC], BF16, tag="b", name=f"b_{s}_{hc}")
            nc.vector.scalar_tensor_tensor(
                bsc, psum_h, bh_sb[:, hc : hc + 1], z_sb, op0=OP.add, op1=OP.mult
            )
            # bf16 scan output: the scan accumulator is fp32 in HW
            # regardless of out dtype, so only stored values round; bf16
            # halves the store bytes.
            sc = scan_p.tile([P, TC], BF16, tag=f"sc{hc}", name=f"sc_{s}_{hc}")
            if tci == 0:
                init = hp_sb[:, b * HC + hc : b * HC + hc + 1]
            else:
                init = scans[s - 1][hc][:, TC - 1 : TC]
            nc.vector.tensor_tensor_scan(sc, a_sb, bsc, init, op0=OP.mult, op1=OP.add)
            scans.setdefault(s, [None] * HC)[hc] = sc
            # Store the scan tile directly in [H, T] layout on the SP
            # HWDGE queue (host transposes). The store's scan-semaphore
            # wait must NOT sit on the ACT engine: it would serialize the
            # sigmoids (which drain the GEMM PSUM banks) behind the DVE
            # scan chain and stall the PE. The SP engine only issues the
            # next-next step's crossbar loads after these, which is safe.
            nc.sync.dma_start(
                out=out[b, ds(hc * P, P), ds(tci * TC, TC)], in_=sc
            )

        # --- steps -------------------------------------------------------
        for s in range(NSTEP):
            if s == 0:
                # GEMMs in z/h quarter-pairs so they track the W quarter
                # DMAs landing on the two queues.
                for r in range(4):
                    pzs = [gemm(0, 2 * r + i, "z") for i in range(2)]
                    phs = [gemm(0, 2 * r + i, "h") for i in range(2)]
                    for i in range(2):
                        post(0, 2 * r + i, pzs[i], phs[i])
            else:
                if s + 1 < NSTEP:
                    t_x(s + 1)
                for hc in range(HC):
                    psum_z = gemm(s, hc, "z")
                    psum_h = gemm(s, hc, "h")
                    post(s, hc, psum_z, psum_h)
                if s - 2 in scans:
                    del scans[s - 2]


def build():
    if "nc" in _CACHE:
        return _CACHE["nc"]
    nc = bacc.Bacc(
        "TRN2", target_bir_lowering=False, debug=False, num_devices=NCORES
    )
    x = nc.dram_tensor("x", [B_LOC, T, D], BF16, kind="ExternalInput").ap()
    h0 = nc.dram_tensor("h0", [B_LOC, H], F32, kind="ExternalInput").ap()
    wzT = nc.dram_tensor("wzT", [D, H], BF16, kind="ExternalInput").ap()
    bz = nc.dram_tensor("bz", [H], F32, kind="ExternalInput").ap()
    whT = nc.dram_tensor("whT", [D, H], BF16, kind="ExternalInput").ap()
    bh = nc.dram_tensor("bh", [H], F32, kind="ExternalInput").ap()
    out = nc.dram_tensor("out", [B_LOC, H, T], BF16, kind="ExternalOutput").ap()
    with tile.TileContext(nc) as tctx:
        _mingru_tile(tctx, out, x, h0, wzT, bz, whT, bh)
    nc.compile()
    _CACHE["nc"] = nc
    return nc


def make_in_maps(x, h_prev, W_z, b_z, W_h, b_h):
    x = np.asarray(x, dtype=np.float32).astype(ml_dtypes.bfloat16)
    h_prev = np.ascontiguousarray(np.asarray(h_prev, dtype=np.float32))
    wzT = np.asarray(W_z, dtype=np.float32).T.astype(ml_dtypes.bfloat16)
    whT = np.asarray(W_h, dtype=np.float32).T.astype(ml_dtypes.bfloat16)
    b_z = np.ascontiguousarray(np.asarray(b_z, dtype=np.float32))
    b_h = np.ascontiguousarray(np.asarray(b_h, dtype=np.float32))
    in_maps = []
    for c in range(NCORES):
        sl = slice(c * B_LOC, (c + 1) * B_LOC)
        in_maps.append(
            {
                "x": np.ascontiguousarray(x[sl]),
                "h0": h_prev[sl],
                "wzT": wzT,
                "bz": b_z,
                "whT": whT,
                "bh": b_h,
            }
        )
    return in_maps


def kernel(x, h_prev, W_z, b_z, W_h, b_h, trace=False):
    nc = build()
    in_maps = make_in_maps(x, h_prev, W_z, b_z, W_h, b_h)
    res = run_bass_kernel_spmd(
        nc, in_maps, core_ids=list(range(NCORES)), trace=trace
    )
    # Device output is [B_loc, H, T] bf16; transpose to (B, T, H) f32 here.
    out = np.concatenate(
        [
            np.asarray(r["out"]).astype(np.float32).transpose(0, 2, 1)
            for r in res.results
        ],
        axis=0,
    )
    if trace:
        _CACHE["last_results"] = res
    return out
